# revision 16
# baseline (speedup 1.0000x reference)
"""Causal self-attention (B=4, T=2048, C=1024, H=16) on 8 Trainium2 NeuronCores.

Sharding: 8 cores = 4 batches x 2 head-groups. Core c handles batch c//2 and
heads 8*(c%2) .. 8*(c%2)+8 (512 of the 1024 channels). Each core computes the
QKV projection for its channels over its batch's 2048 tokens, flash-style
causal attention for its 8 heads, and a partial output projection over its
512 c_proj input rows. The host sums the two partials per batch and adds the
bias terms (b_proj plus the b_v contribution, which is w_proj @ b_v because
softmax rows sum to one; b_k shifts every logit in a row equally so softmax
drops it; b_q and the 1/sqrt(hd) scale are folded into the q weights host-side).

Precision: q/k/x and the QKV matmuls run in fp32r (full fp32 data, ~2 PE
cycles/row) so the softmax logits are accurate; the post-softmax side
(P, V, normalized y, w_proj) runs in bf16 (1 cycle/row) since those errors
average out over the contractions.

Device layout (per core):
  xT   [1024, 2048]  x[b].T, contraction-major for the QKV matmuls
  qkT  [128, 8, 2048] SBUF fp32r: blocks 0-3 = scaled q channels, 4-7 = k
  V    [128, 16, 8, 65] bf16 token-major V per (tok-block, head) with a ones
       column so the PV matmul's row 64 accumulates the softmax denominator
  S^T  [k,q] psum tiles via matmul(lhsT=kT, rhs=qT) (K=64); heads are
       processed in pairs at partition bases 0/64 so their K=64 S matmuls
       row-tile concurrently in the PE array. Exp on ScalarE writes P^T to
       SBUF in bf16; causal masking uses persistent band tiles with
       structurally-zero prefixes plus a triangular multiply on diagonal
       subtiles.
  Denominators bounce through DRAM to repack [1,512] -> [128,4] for a cheap
  batched reciprocal; a K=1 outer-product matmul broadcasts 1/denom across
  the 64 output channels for the normalize multiply, which writes bf16 y.

This container's walrus accepts only one hardware wait slot per instruction,
so after Tile scheduling we split multi-wait sync_info into standalone
EventSemaphore waits (_split_multiwaits).
"""

import sys

if '/opt/trn_rl_repo' not in sys.path:
    sys.path.insert(0, '/opt/trn_rl_repo')

import numpy as np

B, T, C, H = 4, 2048, 1024, 16
HD = C // H            # 64
HPC = 8                # heads per core
CL = HPC * HD          # 512 local channels
NCORES = 8

_cache = {}


def _split_multiwaits(nc, max_waits=1):
    import concourse.mybir as mybir
    n = 0
    ctr = [0]
    for fn in nc.m.functions:
        for bb in fn.blocks:
            out = []
            for inst in bb.instructions:
                si = inst.sync_info
                if si is not None and si.on_wait and len(si.on_wait) > max_waits:
                    waits = list(si.on_wait)
                    head, tail = waits[:-max_waits], waits[-max_waits:]
                    for w in head:
                        ctr[0] += 1
                        out.append(mybir.InstEventSemaphore(
                            name=f"wsplit-{ctr[0]}",
                            engine=inst.engine,
                            ins=[], outs=[],
                            sync_info=mybir.SyncInfo(on_wait=[w], on_update=[]),
                        ))
                    inst.sync_info = mybir.SyncInfo(
                        on_wait=tail, on_update=list(si.on_update))
                    n += 1
                out.append(inst)
            bb.instructions[:] = out
    return n


def _build(reps=1, phases="qkv,attn,norm,proj"):
    phases = set(phases.split(","))
    import contextlib
    import concourse.bass as bass
    import concourse.mybir as mybir
    import concourse.tile as tile

    f32 = mybir.dt.float32
    f32r = mybir.dt.float32r
    bf16 = mybir.dt.bfloat16
    Act = mybir.ActivationFunctionType
    Alu = mybir.AluOpType

    nc = bass.Bass()

    xT_d = nc.dram_tensor("xT", [C, T], f32, kind="ExternalInput")
    wqk_d = nc.dram_tensor("wqk", [C, 2 * CL], f32, kind="ExternalInput")
    wv_d = nc.dram_tensor("wv", [C, CL], f32, kind="ExternalInput")
    bq_d = nc.dram_tensor("bq", [128, 4], f32, kind="ExternalInput")
    wp_d = nc.dram_tensor("wp", [CL, C], bf16, kind="ExternalInput")
    mask_d = nc.dram_tensor("mask", [128, 128], bf16, kind="ExternalInput")
    out_d = nc.dram_tensor("out", [T, C], f32, kind="ExternalOutput")
    dscr_d = nc.dram_tensor("dscr", [32, 512], f32)
    rscr_d = nc.dram_tensor("rscr", [32, 512], f32)

    NQ = T // 512      # 4 q-chunks of 512 tokens
    NT = T // 128      # 16 token-blocks

    with tile.TileContext(nc) as tc:
        with tc.tile_pool(name="persist", bufs=1) as persist, \
             tc.tile_pool(name="xp", bufs=2) as xp, \
             tc.tile_pool(name="wpl", bufs=2) as wpool, \
             tc.tile_pool(name="pt", bufs=4) as ptpool, \
             tc.tile_pool(name="ysb", bufs=6) as ypool, \
             tc.tile_pool(name="outst", bufs=2) as opool, \
             tc.tile_pool(name="rst", bufs=2) as rpool, \
             tc.tile_pool(name="misc", bufs=2) as mpool, \
             tc.tile_pool(name="psmm", bufs=2, space="PSUM") as psmm, \
             tc.tile_pool(name="psst", bufs=3, space="PSUM") as psst, \
             tc.tile_pool(name="psy", bufs=2, space="PSUM") as psy, \
             tc.tile_pool(name="psrep", bufs=1, space="PSUM") as psrep:

            with (tc.For_i(0, reps, 1) if reps > 1 else contextlib.nullcontext()):
                qkT = persist.tile([128, 8, T], f32r, tag="qkT")
                yN = persist.tile([128, 4, T], bf16, tag="yN")
                V = persist.tile([128, NT, HPC, HD + 1], bf16, tag="V")
                bq_sb = persist.tile([128, 4], f32, tag="bq")
                mask_sb = persist.tile([128, 128], bf16, tag="mask")
                ones64 = persist.tile([1, 64], f32r, tag="ones")
                bands = [[persist.tile([128, 512], bf16, tag=f"band{s}{j}",
                                       name=f"band{s}{j}")
                          for j in (1, 2, 3)] for s in (0, 1)]

                nc.sync.dma_start(bq_sb[:], bq_d[:])
                nc.sync.dma_start(mask_sb[:], mask_d[:])
                nc.vector.memset(ones64[:].bitcast(f32), 1.0)
                nc.vector.memset(V[:, :, :, HD:HD + 1], 1.0)
                for s in (0, 1):
                    for jr in (1, 2, 3):
                        nc.vector.memset(bands[s][jr - 1][:, 0:128 * jr], 0.0)

                # ---- QKV projection, one 512-token quarter at a time ----
                for tq in range(4 if "qkv" in phases else 0):
                    ts512 = tq * 512
                    xq = xp.tile([128, 8, 512], f32r, tag="x")
                    nc.sync.dma_start(
                        xq[:],
                        xT_d[:, ts512:ts512 + 512]
                        .rearrange("(j p) n -> p j n", p=128).bitcast(f32r))
                    for half in range(2):
                        wt = wpool.tile([128, 8, 512], f32r, tag="w")
                        nc.sync.dma_start(
                            wt[:],
                            wqk_d[:, half * 512:half * 512 + 512]
                            .rearrange("(j p) m -> p j m", p=128).bitcast(f32r))
                        for mm in range(4):
                            m = half * 4 + mm
                            ps = psmm.tile([128, 512], f32, tag="mm")
                            for j in range(8):
                                nc.tensor.matmul(
                                    ps[:], lhsT=wt[:, j, mm * 128:mm * 128 + 128],
                                    rhs=xq[:, j, :],
                                    start=(j == 0), stop=(j == 7))
                            if m < 4:  # q block: add bias (scale pre-folded)
                                nc.vector.tensor_scalar_add(
                                    qkT[:, m, ts512:ts512 + 512], ps[:],
                                    bq_sb[:, m:m + 1])
                            else:      # k block: plain evacuate
                                nc.vector.tensor_copy(
                                    qkT[:, m, ts512:ts512 + 512], ps[:])
                    wt = wpool.tile([128, 8, 512], f32r, tag="w")
                    nc.sync.dma_start(
                        wt[:],
                        wv_d[:].rearrange("(j p) m -> p j m", p=128).bitcast(f32r))
                    for t in range(4):
                        tb = tq * 4 + t
                        ps = psmm.tile([128, 512], f32, tag="mm")
                        for j in range(8):
                            nc.tensor.matmul(
                                ps[:], lhsT=xq[:, j, t * 128:t * 128 + 128],
                                rhs=wt[:, j, :],
                                start=(j == 0), stop=(j == 7))
                        nc.vector.tensor_copy(
                            V[:, tb, :, 0:HD],
                            ps[:].rearrange("p (h d) -> p h d", h=HPC))

                # ---- causal attention, head-pairs interleaved so the K=64
                # ---- S matmuls at bases 0/64 row-tile concurrently in PE
                for h in range(HPC if "attn" in phases else 0):
                    m = h // 2
                    s = h % 2
                    pb = 64 * s
                    y_store = []
                    for c in range(NQ):
                        cs = c * 512
                        nj = 4 * c + 4
                        y_c = ypool.tile([65, 512], f32, tag="ysb",
                                         name=f"y{h}{c}")
                        ys = psy.tile([65, 512], f32, tag="y")
                        pipe = []
                        for j in range(nj):
                            jrel = j - 4 * c
                            ps_st = psst.tile([128, 512], f32, tag="st")
                            if jrel <= 0:
                                nc.tensor.matmul(
                                    ps_st[:],
                                    lhsT=qkT[pb:pb + 64, 4 + m,
                                             j * 128:j * 128 + 128],
                                    rhs=qkT[pb:pb + 64, m, cs:cs + 512],
                                    start=True, stop=True)
                                pt = ptpool.tile([128, 512], bf16, tag="pt")
                                nc.scalar.activation(pt[:], ps_st[:], Act.Exp)
                            else:
                                z = 128 * jrel
                                nc.tensor.matmul(
                                    ps_st[:, z:512],
                                    lhsT=qkT[pb:pb + 64, 4 + m,
                                             j * 128:j * 128 + 128],
                                    rhs=qkT[pb:pb + 64, m, cs + z:cs + 512],
                                    start=True, stop=True)
                                pt = bands[s][jrel - 1]
                                nc.scalar.activation(
                                    pt[:, z:512], ps_st[:, z:512], Act.Exp)
                            if jrel >= 0:
                                z = 128 * jrel
                                nc.gpsimd.tensor_tensor(
                                    pt[:, z:z + 128], pt[:, z:z + 128],
                                    mask_sb[:], Alu.mult)
                            pipe.append((j, pt))
                            if len(pipe) > 2:
                                pj, ppt = pipe.pop(0)
                                nc.tensor.matmul(
                                    ys[:], lhsT=V[:, pj, h, :], rhs=ppt[:],
                                    start=(pj == 0), stop=False)
                        for pj, ppt in pipe:
                            nc.tensor.matmul(
                                ys[:], lhsT=V[:, pj, h, :], rhs=ppt[:],
                                start=(pj == 0), stop=(pj == nj - 1))
                        nc.vector.tensor_copy(y_c[:], ys[:])
                        y_store.append((c, y_c))

                    # deferred normalization: the DRAM-bounce reciprocal
                    # chains overlap the next head's attention compute
                    if "norm" not in phases:
                        continue
                    for c, y_c in y_store:
                        cs = c * 512
                        hc = h * 4 + c
                        nc.sync.dma_start(dscr_d[hc:hc + 1, :], y_c[64:65, :])
                        dT = mpool.tile([128, 4], f32, tag="dT")
                        nc.sync.dma_start(
                            dT[:],
                            dscr_d[hc, :].rearrange("(i p) -> p i", p=128))
                        rT = mpool.tile([128, 4], f32, tag="rT")
                        nc.vector.reciprocal(rT[:], dT[:])
                        nc.sync.dma_start(
                            rscr_d[hc, :].rearrange("(i p) -> p i", p=128),
                            rT[:])
                        rstage = rpool.tile([1, 512], f32r, tag="rstage")
                        nc.sync.dma_start(rstage[:],
                                          rscr_d[hc:hc + 1, :].bitcast(f32r))
                        ps_rep = psrep.tile([64, 512], f32, tag="rep")
                        nc.tensor.matmul(ps_rep[:], lhsT=ones64[:],
                                         rhs=rstage[:], start=True, stop=True)
                        nc.vector.tensor_tensor(
                            yN[pb:pb + 64, m, cs:cs + 512],
                            y_c[0:64, :], ps_rep[:], Alu.mult)

                # ---- output projection (partial over local channels) ----
                for o in range(2 if "proj" in phases else 0):
                    os_ = o * 512
                    wpt = wpool.tile([128, 4, 512], bf16, tag="wpj")
                    nc.sync.dma_start(
                        wpt[:],
                        wp_d[:, os_:os_ + 512]
                        .rearrange("(mq p) oc -> p mq oc", p=128))
                    for t in range(NT):
                        ps = psmm.tile([128, 512], f32, tag="mm")
                        for mq in range(4):
                            nc.tensor.matmul(
                                ps[:], lhsT=yN[:, mq, t * 128:t * 128 + 128],
                                rhs=wpt[:, mq, :],
                                start=(mq == 0), stop=(mq == 3))
                        ost = opool.tile([128, 512], f32, tag="outst")
                        nc.vector.tensor_copy(ost[:], ps[:])
                        nc.sync.dma_start(
                            out_d[t * 128:t * 128 + 128, os_:os_ + 512], ost[:])

    nsplit = _split_multiwaits(nc)
    return nc, nsplit


def _prep_inputs(x, w_attn, b_attn, w_proj):
    """Per-core input maps. Core c: batch c//2, head-group c%2."""
    import ml_dtypes
    x = np.ascontiguousarray(x, dtype=np.float32)
    w_attn = np.asarray(w_attn, dtype=np.float32)
    b_attn = np.asarray(b_attn, dtype=np.float32)
    w_proj = np.asarray(w_proj, dtype=np.float32)
    scale = 1.0 / np.sqrt(HD)

    mask = (np.arange(128)[:, None] <= np.arange(128)[None, :]).astype(
        ml_dtypes.bfloat16)

    in_maps = []
    for core in range(NCORES):
        b = core // 2
        g = core % 2
        gc = CL * g
        wq = w_attn[gc:gc + CL, :] * scale          # [512, 1024]
        wk = w_attn[C + gc:C + gc + CL, :]
        wv = w_attn[2 * C + gc:2 * C + gc + CL, :]
        bq = b_attn[gc:gc + CL] * scale
        in_maps.append({
            "xT": np.ascontiguousarray(x[b].T),
            "wqk": np.ascontiguousarray(
                np.concatenate([wq.T, wk.T], axis=1)),   # [1024, 1024]
            "wv": np.ascontiguousarray(wv.T),            # [1024, 512]
            "bq": np.ascontiguousarray(bq.reshape(4, 128).T),
            "wp": np.ascontiguousarray(
                w_proj[:, gc:gc + CL].T.astype(ml_dtypes.bfloat16)),
            "mask": mask,
        })
    return in_maps


def _run(in_maps, reps=1):
    from concourse.bass_utils import run_bass_kernel_spmd
    key = reps
    if key not in _cache:
        _cache[key] = _build(reps)
    nc, _ = _cache[key]
    return run_bass_kernel_spmd(nc, in_maps, list(range(NCORES)))


def kernel(x, w_attn, b_attn, w_proj, b_proj):
    x = np.asarray(x, dtype=np.float32)
    w_attn = np.asarray(w_attn, dtype=np.float32)
    b_attn = np.asarray(b_attn, dtype=np.float32)
    w_proj = np.asarray(w_proj, dtype=np.float32)
    b_proj = np.asarray(b_proj, dtype=np.float32)

    in_maps = _prep_inputs(x, w_attn, b_attn, w_proj)
    res = _run(in_maps).results

    # host-side unshard: sum the two head-group partials per batch and add
    # the bias terms (b_proj + w_proj @ b_v; softmax rows sum to 1).
    bv = b_attn[2 * C:]
    const = (w_proj @ bv + b_proj).astype(np.float32)
    out = np.empty((B, T, C), dtype=np.float32)
    for b in range(B):
        out[b] = res[2 * b]["out"] + res[2 * b + 1]["out"] + const
    return out


# revision 17
# speedup vs baseline: 1.1136x; 1.1136x over previous
"""Causal self-attention (B=4, T=2048, C=1024, H=16) on 8 Trainium2 NeuronCores.

Sharding: 8 cores = 4 batches x 2 head-groups. Core c handles batch c//2 and
heads 8*(c%2) .. 8*(c%2)+8 (512 of the 1024 channels). Each core computes the
QKV projection for its channels over its batch's 2048 tokens, flash-style
causal attention for its 8 heads, and a partial output projection over its
512 c_proj input rows. The host sums the two partials per batch and adds the
bias terms (b_proj plus the b_v contribution, which is w_proj @ b_v because
softmax rows sum to one; b_k shifts every logit in a row equally so softmax
drops it; b_q and the 1/sqrt(hd) scale are folded into the q weights host-side).

Precision: q/k/x and the QKV matmuls run in fp32r (full fp32 data, ~2 PE
cycles/row) so the softmax logits are accurate; the post-softmax side
(P, V, normalized y, w_proj) runs in bf16 (1 cycle/row) since those errors
average out over the contractions.

Device layout (per core):
  xT   [1024, 2048]  x[b].T, contraction-major for the QKV matmuls
  qkT  [128, 8, 2048] SBUF fp32r: blocks 0-3 = scaled q channels, 4-7 = k
  V    [128, 16, 8, 65] bf16 token-major V per (tok-block, head) with a ones
       column so the PV matmul's row 64 accumulates the softmax denominator
  S^T  [k,q] psum tiles via matmul(lhsT=kT, rhs=qT) (K=64); heads are
       processed in pairs at partition bases 0/64 so their K=64 S matmuls
       row-tile concurrently in the PE array. Exp on ScalarE writes P^T to
       SBUF in bf16; causal masking uses persistent band tiles with
       structurally-zero prefixes plus a triangular multiply on diagonal
       subtiles.
  Denominators bounce through DRAM to repack [1,512] -> [128,4] for a cheap
  batched reciprocal; a K=1 outer-product matmul broadcasts 1/denom across
  the 64 output channels for the normalize multiply, which writes bf16 y.

This container's walrus accepts only one hardware wait slot per instruction,
so after Tile scheduling we split multi-wait sync_info into standalone
EventSemaphore waits (_split_multiwaits).
"""

import sys

if '/opt/trn_rl_repo' not in sys.path:
    sys.path.insert(0, '/opt/trn_rl_repo')

import numpy as np

B, T, C, H = 4, 2048, 1024, 16
HD = C // H            # 64
HPC = 8                # heads per core
CL = HPC * HD          # 512 local channels
NCORES = 8

_cache = {}


def _split_multiwaits(nc, max_waits=1):
    import concourse.mybir as mybir
    n = 0
    ctr = [0]
    for fn in nc.m.functions:
        for bb in fn.blocks:
            out = []
            for inst in bb.instructions:
                si = inst.sync_info
                if si is not None and si.on_wait and len(si.on_wait) > max_waits:
                    waits = list(si.on_wait)
                    head, tail = waits[:-max_waits], waits[-max_waits:]
                    for w in head:
                        ctr[0] += 1
                        out.append(mybir.InstEventSemaphore(
                            name=f"wsplit-{ctr[0]}",
                            engine=inst.engine,
                            ins=[], outs=[],
                            sync_info=mybir.SyncInfo(on_wait=[w], on_update=[]),
                        ))
                    inst.sync_info = mybir.SyncInfo(
                        on_wait=tail, on_update=list(si.on_update))
                    n += 1
                out.append(inst)
            bb.instructions[:] = out
    return n


def _build(reps=1, phases="qkv,attn,norm,proj"):
    phases = set(phases.split(","))
    import contextlib
    import concourse.bass as bass
    import concourse.mybir as mybir
    import concourse.tile as tile

    f32 = mybir.dt.float32
    f32r = mybir.dt.float32r
    bf16 = mybir.dt.bfloat16
    Act = mybir.ActivationFunctionType
    Alu = mybir.AluOpType

    nc = bass.Bass()

    xT_d = nc.dram_tensor("xT", [C, T], f32, kind="ExternalInput")
    wqk_d = nc.dram_tensor("wqk", [C, 2 * CL], f32, kind="ExternalInput")
    wv_d = nc.dram_tensor("wv", [C, CL], f32, kind="ExternalInput")
    bq_d = nc.dram_tensor("bq", [128, 4], f32, kind="ExternalInput")
    wp_d = nc.dram_tensor("wp", [CL, C], bf16, kind="ExternalInput")
    mask_d = nc.dram_tensor("mask", [128, 128], bf16, kind="ExternalInput")
    out_d = nc.dram_tensor("out", [T, C], f32, kind="ExternalOutput")
    dscr_d = nc.dram_tensor("dscr", [32, 512], f32)
    rscr_d = nc.dram_tensor("rscr", [32, 512], f32)

    NQ = T // 512      # 4 q-chunks of 512 tokens
    NT = T // 128      # 16 token-blocks

    with tile.TileContext(nc) as tc:
        with tc.tile_pool(name="persist", bufs=1) as persist, \
             tc.tile_pool(name="xp", bufs=2) as xp, \
             tc.tile_pool(name="wpl", bufs=2) as wpool, \
             tc.tile_pool(name="pt", bufs=4) as ptpool, \
             tc.tile_pool(name="ysb", bufs=6) as ypool, \
             tc.tile_pool(name="outst", bufs=2) as opool, \
             tc.tile_pool(name="rst", bufs=2) as rpool, \
             tc.tile_pool(name="misc", bufs=2) as mpool, \
             tc.tile_pool(name="psmm", bufs=2, space="PSUM") as psmm, \
             tc.tile_pool(name="psst", bufs=3, space="PSUM") as psst, \
             tc.tile_pool(name="psy", bufs=2, space="PSUM") as psy, \
             tc.tile_pool(name="psrep", bufs=1, space="PSUM") as psrep:

            with (tc.For_i(0, reps, 1) if reps > 1 else contextlib.nullcontext()):
                qkT = persist.tile([128, 8, T], f32r, tag="qkT")
                yN = persist.tile([128, 4, T], bf16, tag="yN")
                V = persist.tile([128, NT, HPC, HD + 1], bf16, tag="V")
                bq_sb = persist.tile([128, 4], f32, tag="bq")
                mask_sb = persist.tile([128, 128], bf16, tag="mask")
                ones64 = persist.tile([1, 64], f32r, tag="ones")
                bands = [[persist.tile([128, 512], bf16, tag=f"band{s}{j}",
                                       name=f"band{s}{j}")
                          for j in (1, 2, 3)] for s in (0, 1)]

                nc.sync.dma_start(bq_sb[:], bq_d[:])
                nc.sync.dma_start(mask_sb[:], mask_d[:])
                nc.vector.memset(ones64[:].bitcast(f32), 1.0)
                nc.vector.memset(V[:, :, :, HD:HD + 1], 1.0)
                for s in (0, 1):
                    for jr in (1, 2, 3):
                        nc.vector.memset(bands[s][jr - 1][:, 0:128 * jr], 0.0)

                # ---- QKV projection, one 512-token quarter at a time ----
                for tq in range(4 if "qkv" in phases else 0):
                    ts512 = tq * 512
                    xq = xp.tile([128, 8, 512], f32r, tag="x")
                    nc.sync.dma_start(
                        xq[:],
                        xT_d[:, ts512:ts512 + 512]
                        .rearrange("(j p) n -> p j n", p=128).bitcast(f32r))
                    for half in range(2):
                        wt = wpool.tile([128, 8, 512], f32r, tag="w")
                        nc.sync.dma_start(
                            wt[:],
                            wqk_d[:, half * 512:half * 512 + 512]
                            .rearrange("(j p) m -> p j m", p=128).bitcast(f32r))
                        for mm in range(4):
                            m = half * 4 + mm
                            ps = psmm.tile([128, 512], f32, tag="mm")
                            for j in range(8):
                                nc.tensor.matmul(
                                    ps[:], lhsT=wt[:, j, mm * 128:mm * 128 + 128],
                                    rhs=xq[:, j, :],
                                    start=(j == 0), stop=(j == 7))
                            if m < 4:  # q block: add bias (scale pre-folded)
                                nc.vector.tensor_scalar_add(
                                    qkT[:, m, ts512:ts512 + 512], ps[:],
                                    bq_sb[:, m:m + 1])
                            else:      # k block: plain evacuate
                                nc.vector.tensor_copy(
                                    qkT[:, m, ts512:ts512 + 512], ps[:])
                    wt = wpool.tile([128, 8, 512], f32r, tag="w")
                    nc.sync.dma_start(
                        wt[:],
                        wv_d[:].rearrange("(j p) m -> p j m", p=128).bitcast(f32r))
                    for t in range(4):
                        tb = tq * 4 + t
                        ps = psmm.tile([128, 512], f32, tag="mm")
                        for j in range(8):
                            nc.tensor.matmul(
                                ps[:], lhsT=xq[:, j, t * 128:t * 128 + 128],
                                rhs=wt[:, j, :],
                                start=(j == 0), stop=(j == 7))
                        nc.vector.tensor_copy(
                            V[:, tb, :, 0:HD],
                            ps[:].rearrange("p (h d) -> p h d", h=HPC))

                # ---- causal attention, head-pairs interleaved so the K=64
                # ---- S matmuls at bases 0/64 row-tile concurrently in PE
                for h in range(HPC if "attn" in phases else 0):
                    m = h // 2
                    s = h % 2
                    pb = 64 * s
                    y_store = []
                    for c in range(NQ):
                        cs = c * 512
                        nj = 4 * c + 4
                        y_c = ypool.tile([65, 512], f32, tag="ysb",
                                         name=f"y{h}{c}")
                        ys = psy.tile([65, 512], f32, tag="y")
                        prev = None
                        for j in range(nj):
                            jrel = j - 4 * c
                            ps_st = psst.tile([128, 512], f32, tag="st")
                            nc.tensor.matmul(
                                ps_st[:],
                                lhsT=qkT[pb:pb + 64, 4 + m,
                                         j * 128:j * 128 + 128],
                                rhs=qkT[pb:pb + 64, m, cs:cs + 512],
                                start=True, stop=True)
                            if jrel <= 0:
                                pt = ptpool.tile([128, 512], bf16, tag="pt")
                                nc.scalar.activation(pt[:], ps_st[:], Act.Exp)
                            else:
                                pt = bands[s][jrel - 1]
                                z = 128 * jrel
                                nc.scalar.activation(
                                    pt[:, z:512], ps_st[:, z:512], Act.Exp)
                            if jrel >= 0:
                                z = 128 * jrel
                                nc.gpsimd.tensor_tensor(
                                    pt[:, z:z + 128], pt[:, z:z + 128],
                                    mask_sb[:], Alu.mult)
                            if prev is not None:
                                pj, ppt = prev
                                nc.tensor.matmul(
                                    ys[:], lhsT=V[:, pj, h, :], rhs=ppt[:],
                                    start=(pj == 0), stop=False)
                            prev = (j, pt)
                        pj, ppt = prev
                        nc.tensor.matmul(
                            ys[:], lhsT=V[:, pj, h, :], rhs=ppt[:],
                            start=(pj == 0), stop=True)
                        nc.vector.tensor_copy(y_c[:], ys[:])
                        y_store.append((c, y_c))

                    # deferred normalization: the DRAM-bounce reciprocal
                    # chains overlap the next head's attention compute
                    if "norm" not in phases:
                        continue
                    for c, y_c in y_store:
                        cs = c * 512
                        hc = h * 4 + c
                        nc.sync.dma_start(dscr_d[hc:hc + 1, :], y_c[64:65, :])
                        dT = mpool.tile([128, 4], f32, tag="dT")
                        nc.sync.dma_start(
                            dT[:],
                            dscr_d[hc, :].rearrange("(i p) -> p i", p=128))
                        rT = mpool.tile([128, 4], f32, tag="rT")
                        nc.vector.reciprocal(rT[:], dT[:])
                        nc.sync.dma_start(
                            rscr_d[hc, :].rearrange("(i p) -> p i", p=128),
                            rT[:])
                        rstage = rpool.tile([1, 512], f32r, tag="rstage")
                        nc.sync.dma_start(rstage[:],
                                          rscr_d[hc:hc + 1, :].bitcast(f32r))
                        ps_rep = psrep.tile([64, 512], f32, tag="rep")
                        nc.tensor.matmul(ps_rep[:], lhsT=ones64[:],
                                         rhs=rstage[:], start=True, stop=True)
                        nc.vector.tensor_tensor(
                            yN[pb:pb + 64, m, cs:cs + 512],
                            y_c[0:64, :], ps_rep[:], Alu.mult)

                # ---- output projection (partial over local channels) ----
                for o in range(2 if "proj" in phases else 0):
                    os_ = o * 512
                    wpt = wpool.tile([128, 4, 512], bf16, tag="wpj")
                    nc.sync.dma_start(
                        wpt[:],
                        wp_d[:, os_:os_ + 512]
                        .rearrange("(mq p) oc -> p mq oc", p=128))
                    for t in range(NT):
                        ps = psmm.tile([128, 512], f32, tag="mm")
                        for mq in range(4):
                            nc.tensor.matmul(
                                ps[:], lhsT=yN[:, mq, t * 128:t * 128 + 128],
                                rhs=wpt[:, mq, :],
                                start=(mq == 0), stop=(mq == 3))
                        ost = opool.tile([128, 512], f32, tag="outst")
                        nc.vector.tensor_copy(ost[:], ps[:])
                        nc.sync.dma_start(
                            out_d[t * 128:t * 128 + 128, os_:os_ + 512], ost[:])

    nsplit = _split_multiwaits(nc)
    return nc, nsplit


def _prep_inputs(x, w_attn, b_attn, w_proj):
    """Per-core input maps. Core c: batch c//2, head-group c%2."""
    import ml_dtypes
    x = np.ascontiguousarray(x, dtype=np.float32)
    w_attn = np.asarray(w_attn, dtype=np.float32)
    b_attn = np.asarray(b_attn, dtype=np.float32)
    w_proj = np.asarray(w_proj, dtype=np.float32)
    scale = 1.0 / np.sqrt(HD)

    mask = (np.arange(128)[:, None] <= np.arange(128)[None, :]).astype(
        ml_dtypes.bfloat16)

    in_maps = []
    for core in range(NCORES):
        b = core // 2
        g = core % 2
        gc = CL * g
        wq = w_attn[gc:gc + CL, :] * scale          # [512, 1024]
        wk = w_attn[C + gc:C + gc + CL, :]
        wv = w_attn[2 * C + gc:2 * C + gc + CL, :]
        bq = b_attn[gc:gc + CL] * scale
        in_maps.append({
            "xT": np.ascontiguousarray(x[b].T),
            "wqk": np.ascontiguousarray(
                np.concatenate([wq.T, wk.T], axis=1)),   # [1024, 1024]
            "wv": np.ascontiguousarray(wv.T),            # [1024, 512]
            "bq": np.ascontiguousarray(bq.reshape(4, 128).T),
            "wp": np.ascontiguousarray(
                w_proj[:, gc:gc + CL].T.astype(ml_dtypes.bfloat16)),
            "mask": mask,
        })
    return in_maps


def _run(in_maps, reps=1):
    from concourse.bass_utils import run_bass_kernel_spmd
    key = reps
    if key not in _cache:
        _cache[key] = _build(reps)
    nc, _ = _cache[key]
    return run_bass_kernel_spmd(nc, in_maps, list(range(NCORES)))


def kernel(x, w_attn, b_attn, w_proj, b_proj):
    x = np.asarray(x, dtype=np.float32)
    w_attn = np.asarray(w_attn, dtype=np.float32)
    b_attn = np.asarray(b_attn, dtype=np.float32)
    w_proj = np.asarray(w_proj, dtype=np.float32)
    b_proj = np.asarray(b_proj, dtype=np.float32)

    in_maps = _prep_inputs(x, w_attn, b_attn, w_proj)
    res = _run(in_maps).results

    # host-side unshard: sum the two head-group partials per batch and add
    # the bias terms (b_proj + w_proj @ b_v; softmax rows sum to 1).
    bv = b_attn[2 * C:]
    const = (w_proj @ bv + b_proj).astype(np.float32)
    out = np.empty((B, T, C), dtype=np.float32)
    for b in range(B):
        out[b] = res[2 * b]["out"] + res[2 * b + 1]["out"] + const
    return out
